# revision 9
# baseline (speedup 1.0000x reference)
"""Trainium2 Bass kernel for single-head causal attention.

Problem: B=4, S=2048, E=1024, H=64 fp32.
  q = x@Wq+bq; k = x@Wk+bk; v = x@Wv+bv
  out = softmax(causal(q k^T / sqrt(H))) v

Sharding (V0): 8 cores, core c processes batch c//2 fully (pairs are
redundant; host reads even cores). Inside a core everything runs in a
transposed "head-dim on partitions" layout:

  x^T tiles  [e=128, s=512]  via PE transposes (x DMA'd naturally)
  QT|KT      [64+64, s]      = (Wq|Wk)-chunk stationary @ x^T moving
  VT         [64, s]         = Wv-chunk stationary @ x^T, PE-transposed
                              to V blocks [k=128, 65] with ones column
  scores^T   [k=128, q=512]  = KT-block stationary @ QT moving (contract h)
  p = exp(scoresT*0.125)     ACT; no max subtraction (scores are O(5))
  diagonal blocks: p *= 0/1 ramp-mask slice (causality)
  pv         [65, q=512]     += V-block stationary @ p (contract k);
                              row 64 = softmax denominator (free)
  out tile   [q=128, 65]     PE transpose of pv; y = pv[:,0:64]/pv[:,64]
"""

import sys
from contextlib import ExitStack

import numpy as np

if "/opt/trn_rl_repo" not in sys.path:
    sys.path.insert(0, "/opt/trn_rl_repo")

import concourse.bacc as bacc
import concourse.mybir as mybir
import concourse.tile as tile

B, S, E, H = 4, 2048, 1024, 64
NCORES = 8
F32 = mybir.dt.float32
AF = mybir.ActivationFunctionType

ST = 512          # s-tile width for projections
NST = S // ST     # 4 s-tiles
NEC = E // 128    # 8 e-chunks (contraction)
QW = 512          # q-tile width in attention
NQT = S // QW     # 4 q-tiles
NKB = S // 128    # 16 total key blocks


def build_program():
    nc = bacc.Bacc("TRN2", target_bir_lowering=False, debug=False,
                   num_devices=NCORES)

    x_d = nc.dram_tensor("x", [S, E], F32, kind="ExternalInput")
    wqk_d = nc.dram_tensor("wqk", [E, 128], F32, kind="ExternalInput")
    wv_d = nc.dram_tensor("wv", [E, H], F32, kind="ExternalInput")
    bqk_d = nc.dram_tensor("bqk", [128, 1], F32, kind="ExternalInput")
    bv_d = nc.dram_tensor("bv", [H, 1], F32, kind="ExternalInput")
    id_d = nc.dram_tensor("ident", [128, 128], F32, kind="ExternalInput")
    w2_d = nc.dram_tensor("w2", [128, 1024], F32, kind="ExternalInput")
    y_d = nc.dram_tensor("y", [S, H], F32, kind="ExternalOutput")

    with tile.TileContext(nc) as tc, ExitStack() as ctx:
        singles = ctx.enter_context(tc.tile_pool(name="singles", bufs=1))
        xpool = ctx.enter_context(tc.tile_pool(name="xpool", bufs=4))
        xtpool = ctx.enter_context(tc.tile_pool(name="xtpool", bufs=2))
        vtpool = ctx.enter_context(tc.tile_pool(name="vtpool", bufs=2))
        ppool = ctx.enter_context(tc.tile_pool(name="ppool", bufs=4))
        opool = ctx.enter_context(tc.tile_pool(name="opool", bufs=8))
        # PSUM: 8 banks total. Tags: big(2) + p65(2) + small(3) = 7 banks.
        psA = ctx.enter_context(tc.tile_pool(name="psA", bufs=2, space="PSUM"))
        psB = ctx.enter_context(tc.tile_pool(name="psB", bufs=2, space="PSUM"))
        psC = ctx.enter_context(tc.tile_pool(name="psC", bufs=3, space="PSUM"))

        # ---- constants / persistent tensors ----
        ident = singles.tile([128, 128], F32)
        nc.sync.dma_start(out=ident, in_=id_d[:, :])
        w2 = singles.tile([128, 1024], F32)
        nc.sync.dma_start(out=w2, in_=w2_d[:, :])
        bqk = singles.tile([128, 1], F32)
        nc.sync.dma_start(out=bqk, in_=bqk_d[:, :])
        bv = singles.tile([H, 1], F32)
        nc.sync.dma_start(out=bv, in_=bv_d[:, :])

        wqk = singles.tile([128, NEC, 128], F32)
        nc.sync.dma_start(
            out=wqk, in_=wqk_d.ap().rearrange("(c p) m -> p c m", p=128))
        wv = singles.tile([128, NEC, H], F32)
        nc.sync.dma_start(
            out=wv, in_=wv_d.ap().rearrange("(c p) m -> p c m", p=128))

        qt_all = singles.tile([64, S], F32)    # Q^T, h on partitions
        kt_all = singles.tile([64, S], F32)    # K^T
        v_all = singles.tile([128, NKB, H + 1], F32)  # V blocks + ones col
        nc.vector.memset(v_all[:, :, H:H + 1], 1.0)

        # ---- phase 1: transpose x, project QT/KT/VT, build V blocks ----
        for st in range(NST):
            xts = []
            for ec in range(NEC):
                xts.append(xtpool.tile([128, ST], F32, tag=f"xt{ec}",
                                       name=f"xt{ec}_{st}"))
            for sb in range(ST // 128):
                xn = xpool.tile([128, E], F32, tag="xn")
                nc.sync.dma_start(
                    out=xn, in_=x_d[st * ST + sb * 128: st * ST + (sb + 1) * 128, :])
                for ec in range(NEC):
                    pt = psC.tile([128, 128], F32, tag="small")
                    nc.tensor.transpose(pt, xn[:, ec * 128:(ec + 1) * 128], ident)
                    nc.vector.tensor_copy(
                        xts[ec][:, sb * 128:(sb + 1) * 128], pt)

            pqk = psA.tile([128, ST], F32, tag="big")
            for ec in range(NEC):
                nc.tensor.matmul(pqk, wqk[:, ec, :], xts[ec],
                                 start=(ec == 0), stop=(ec == NEC - 1))
            nc.scalar.activation(qt_all[:, st * ST:(st + 1) * ST],
                                 pqk[0:64, :], AF.Identity, bias=bqk[0:64, :])
            nc.scalar.activation(kt_all[:, st * ST:(st + 1) * ST],
                                 pqk[64:128, :], AF.Identity, bias=bqk[64:128, :])

            pvt = psB.tile([H + 1, ST], F32, tag="p65")
            for ec in range(NEC):
                nc.tensor.matmul(pvt[0:H, :], wv[:, ec, :], xts[ec],
                                 start=(ec == 0), stop=(ec == NEC - 1))
            vt = vtpool.tile([H, ST], F32, tag="vt")
            nc.scalar.activation(vt, pvt[0:H, :], AF.Identity, bias=bv)
            for sb in range(ST // 128):
                pv = psC.tile([128, 128], F32, tag="small")
                nc.tensor.transpose(pv[:, 0:H], vt[:, sb * 128:(sb + 1) * 128],
                                    ident[0:H, 0:H])
                nc.vector.tensor_copy(
                    v_all[:, st * (ST // 128) + sb, 0:H], pv[:, 0:H])

        # ---- phase 2: attention ----
        for qt in range(NQT):
            nkb = 4 * (qt + 1)
            ppv = psB.tile([H + 1, QW], F32, tag="p65")
            for kb in range(nkb):
                ps = psA.tile([128, QW], F32, tag="big")
                nc.tensor.matmul(ps, kt_all[:, kb * 128:(kb + 1) * 128],
                                 qt_all[:, qt * QW:(qt + 1) * QW],
                                 start=True, stop=True)
                p_sb = ppool.tile([128, QW], F32, tag="p")
                nc.scalar.activation(p_sb, ps, AF.Exp, scale=0.125)
                if kb >= 4 * qt:
                    d = kb * 128 - qt * QW
                    nc.vector.tensor_mul(p_sb, p_sb, w2[:, 512 - d:1024 - d])
                nc.tensor.matmul(ppv, v_all[:, kb, :], p_sb,
                                 start=(kb == 0), stop=(kb == nkb - 1))
            pv_sb = ppool.tile([H + 1, QW], F32, tag="pv_sb")
            nc.scalar.copy(pv_sb, ppv)
            for j in range(QW // 128):
                po = psC.tile([128, 128], F32, tag="small")
                nc.tensor.transpose(po[:, 0:H + 1],
                                    pv_sb[:, j * 128:(j + 1) * 128],
                                    ident[0:H + 1, 0:H + 1])
                rec = opool.tile([128, 1], F32, tag="rec")
                nc.vector.reciprocal(rec, po[:, H:H + 1])
                o_sb = opool.tile([128, H], F32, tag="o")
                nc.vector.tensor_scalar_mul(o_sb, po[:, 0:H], rec)
                nc.sync.dma_start(
                    out=y_d[qt * QW + j * 128: qt * QW + (j + 1) * 128, :],
                    in_=o_sb)

    nc.compile()
    return nc


_NC_CACHE = None


def _get_nc():
    global _NC_CACHE
    if _NC_CACHE is None:
        _NC_CACHE = build_program()
    return _NC_CACHE


def make_host_inputs(x, Wq, bq, Wk, bk, Wv, bv):
    """Per-core input maps from the full problem inputs."""
    x = np.asarray(x, np.float32)
    wqk = np.hstack([np.asarray(Wq, np.float32), np.asarray(Wk, np.float32)])
    wv = np.asarray(Wv, np.float32)
    bqk = np.concatenate([np.asarray(bq, np.float32),
                          np.asarray(bk, np.float32)]).reshape(128, 1)
    bvv = np.asarray(bv, np.float32).reshape(H, 1)
    ident = np.eye(128, dtype=np.float32)
    # w2[p, g] = 1 iff g >= p + 512 ; slice [512-d : 1024-d] gives
    # mask[p, f] = 1 iff f >= p + d
    gg = np.arange(1024)[None, :]
    pp = np.arange(128)[:, None]
    w2 = (gg >= pp + 512).astype(np.float32)
    maps = []
    for c in range(NCORES):
        maps.append({
            "x": np.ascontiguousarray(x[c // 2]),
            "wqk": wqk, "wv": wv, "bqk": bqk, "bv": bvv,
            "ident": ident, "w2": w2,
        })
    return maps


def run_cores(in_maps, trace=False):
    from concourse.bass_utils import run_bass_kernel_spmd
    nc = _get_nc()
    return run_bass_kernel_spmd(nc, in_maps, list(range(NCORES)), trace=trace)


def kernel(x, Wq, bq, Wk, bk, Wv, bv):
    in_maps = make_host_inputs(x, Wq, bq, Wk, bk, Wv, bv)
    res = run_cores(in_maps).results
    out = np.stack([res[2 * b]["y"] for b in range(B)])
    return out.astype(np.float32)
